# revision 34
# baseline (speedup 1.0000x reference)
"""Trainium2 Bass kernel for nn_AuxiliaryLoss (FAPE + torsion auxiliary loss).

Strategy
--------
dist^2[l,b,i,j] = |Rp_i^T(u_j-u_i) - Rt_i^T(v_j-v_i)|^2 factorizes exactly as a
rank-28 inner product  L_i . R_j  with per-residue factors:
  quadratic blocks: Gp=RpRp^T (sym, 6), Gt=RtRt^T (sym, 6), M=RpRt^T (9)
  linear blocks:    2(c-g).u_j (3), 2(d-h).v_j (3), bias_i (1x1)
so the O(N^2) pairwise tensor is a K=28 matmul per (l,b). Factors are built on
host (O(L*B*N) work), split hi/lo into fp16 (10+10 mantissa bits) and the
full product (Lh+Ll)@(Rh+Rl) is computed as ONE K=112 matmul by concatenating
the four cross-products along the contraction dim (matmul cost is
K-independent), giving near-fp32 accuracy at fp16 speed.
Then: ScalarE sqrt(d2+EPS+guard) PSUM->SBUF, VectorE min(.,10) with fused
per-partition row-sum accumulate, and a final PE reduction over partitions.
The torsion-angle loss (O(L*B*N*7)) runs on-device on VectorE/ScalarE.

Sharding: layer l (L=8) <-> NeuronCore (8 cores), no collectives; host sums
the per-layer partial losses.
"""

import numpy as np

L, B, N = 8, 4, 1024
NT = N // 128  # 8 i-tiles of 128
NJ = 2         # 2 j-tiles of 512
KF = 28        # factor rank
EPS = 1e-4
GUARD = 1e-3   # sqrt guard: ~8x the worst fp16-split d2 error (~1.2e-4)
D_CLAMP = 10.0
Z = 10.0

CHI_MASK_TABLE = np.array([
    [0.,0.,0.,0.], [1.,1.,1.,1.], [1.,1.,0.,0.], [1.,1.,0.,0.],
    [1.,0.,0.,0.], [1.,1.,1.,0.], [1.,1.,1.,0.], [0.,0.,0.,0.],
    [1.,1.,0.,0.], [1.,1.,0.,0.], [1.,1.,0.,0.], [1.,1.,1.,1.],
    [1.,1.,1.,0.], [1.,1.,0.,0.], [1.,1.,0.,0.], [1.,0.,0.,0.],
    [1.,0.,0.,0.], [1.,1.,0.,0.], [1.,1.,0.,0.], [1.,0.,0.,0.],
    [0.,0.,0.,0.],
], dtype=np.float64)

_NC_CACHE = {}
LAST_RESULTS = None  # BassKernelResults of the most recent device run


# --------------------------------------------------------------------------
# host-side factor construction (float64, cast at the end)
# --------------------------------------------------------------------------

def _bf16_split(x32):
    hi = x32.astype(np.float16)
    lo = (x32 - hi.astype(np.float32)).astype(np.float16)
    return hi, lo


def _perm_nt(x, trailing):
    """(B, N, *trailing) -> (128, B*8*prod(trailing)) with p = n % 128."""
    t = int(np.prod(trailing)) if trailing else 1
    return (
        x.reshape(B, NT, 128, t)
        .transpose(2, 0, 1, 3)
        .reshape(128, B * NT * t)
    )


def _build_host_data(traj_rotations, traj_translations, traj_torsion_angles,
                     true_rotations, true_translations, true_torsion_angles,
                     true_torsion_angles_alt, res_types, seq_mask):
    f8 = np.float64
    Rp = traj_rotations.astype(f8)          # (L,B,N,3,3)
    u = traj_translations.astype(f8)        # (L,B,N,3)
    Rt = true_rotations.astype(f8)          # (B,N,3,3)
    v = true_translations.astype(f8)        # (B,N,3)

    Gp = np.einsum('lbnpo,lbnqo->lbnpq', Rp, Rp)
    Gt = np.einsum('bnpo,bnqo->bnpq', Rt, Rt)
    M = np.einsum('lbnpo,bnqo->lbnpq', Rp, Rt)
    g = np.einsum('lbnpq,lbnq->lbnp', Gp, u)
    h = np.einsum('bnpq,bnq->bnp', Gt, v)
    c = np.einsum('lbnpq,bnq->lbnp', M, v)
    d = np.einsum('lbnpq,lbnp->lbnq', M, u)
    s = np.einsum('lbnp,lbnp->lbn', u, c)
    bias = (np.einsum('lbnp,lbnp->lbn', u, g)
            + np.einsum('bnp,bnp->bn', v, h)[None] - 2.0 * s)

    Lfac = np.empty((L, B, N, KF), f8)
    Rfac = np.empty((L, B, N, KF), f8)
    od = [(0, 1), (0, 2), (1, 2)]
    for k in range(3):
        Lfac[..., k] = Gp[..., k, k]
        Rfac[..., k] = u[..., k] * u[..., k]
        p, q = od[k]
        Lfac[..., 3 + k] = 2.0 * Gp[..., p, q]
        Rfac[..., 3 + k] = u[..., p] * u[..., q]
        Lfac[..., 6 + k] = Gt[None, ..., k, k]
        Rfac[..., 6 + k] = (v[..., k] * v[..., k])[None]
        Lfac[..., 9 + k] = 2.0 * Gt[None, ..., p, q]
        Rfac[..., 9 + k] = (v[..., p] * v[..., q])[None]
    Lfac[..., 12:21] = -2.0 * M.reshape(L, B, N, 9)
    Rfac[..., 12:21] = np.einsum('lbnp,bnq->lbnpq', u, v).reshape(L, B, N, 9)
    Lfac[..., 21:24] = 2.0 * (c - g)
    Rfac[..., 21:24] = u
    Lfac[..., 24:27] = 2.0 * (d - h[None])
    Rfac[..., 24:27] = v[None]
    Lfac[..., 27] = bias
    Rfac[..., 27] = 1.0

    # -> (L, KF, B, N) transposed factor layouts
    LfT = Lfac.transpose(0, 3, 1, 2).astype(np.float32)   # (L,28,B,N)
    RfT = Rfac.transpose(0, 3, 1, 2).astype(np.float32)
    Lh, Ll = _bf16_split(LfT)
    Rh, Rl = _bf16_split(RfT)

    # K-concatenated split-product: (Lh+Ll)@(Rh+Rl) as one K=4*KF matmul
    lhs = np.zeros((L, 4 * KF, B * N), np.float16)
    rhs = np.zeros((L, 4 * KF, B * N), np.float16)
    lhs[:, 0 * KF:1 * KF] = Lh.reshape(L, KF, B * N)
    lhs[:, 1 * KF:2 * KF] = Lh.reshape(L, KF, B * N)
    lhs[:, 2 * KF:3 * KF] = Ll.reshape(L, KF, B * N)
    lhs[:, 3 * KF:4 * KF] = Ll.reshape(L, KF, B * N)
    rhs[:, 0 * KF:1 * KF] = Rh.reshape(L, KF, B * N)
    rhs[:, 1 * KF:2 * KF] = Rl.reshape(L, KF, B * N)
    rhs[:, 2 * KF:3 * KF] = Rh.reshape(L, KF, B * N)
    rhs[:, 3 * KF:4 * KF] = Rl.reshape(L, KF, B * N)

    # masks / scales
    m = seq_mask.astype(f8)                                  # (B,N)
    pair_count = np.maximum((m.sum(1)) ** 2, 1.0)            # (B,)
    scale_fape = 1.0 / (Z * pair_count * L)                  # (B,)
    mask_exp = np.empty((128, 32), np.float32)
    for b in range(B):
        for it in range(NT):
            mask_exp[:, b * 8 + it] = m[b, it * 128:(it + 1) * 128] * scale_fape[b]

    # ---- torsion host data ----
    t = traj_torsion_angles.astype(f8)        # (L,B,N,7,2)
    T = true_torsion_angles.astype(f8)        # (B,N,7,2)
    A = true_torsion_angles_alt.astype(f8)

    chi = CHI_MASK_TABLE[res_types]                          # (B,N,4)
    tmask = np.concatenate([np.ones_like(chi[..., :3]), chi], -1)  # (B,N,7)
    tmask = tmask * m[..., None]
    normalizer = np.maximum(tmask.sum((1, 2)), 1.0)          # (B,)
    tmn = tmask / (normalizer[:, None, None] * L)
    tm02 = 0.02 * tmn

    pt1 = (T ** 2).sum(-1) + 1.0                             # (B,N,7)
    pa1 = (A ** 2).sum(-1) + 1.0

    tta = np.stack([_perm_nt(t[l].astype(np.float32), (7, 2)) for l in range(L)])
    tt_sb = _perm_nt(T.astype(np.float32), (7, 2))           # (128,448)
    ta_sb = _perm_nt(A.astype(np.float32), (7, 2))
    pt1_sb = _perm_nt(pt1.astype(np.float32), (7,))          # (128,224)
    pa1_sb = _perm_nt(pa1.astype(np.float32), (7,))
    tmn_sb = _perm_nt(tmn.astype(np.float32), (7,))
    tm02_sb = _perm_nt(tm02.astype(np.float32), (7,))

    aux_common = np.concatenate(
        [tt_sb, ta_sb, pt1_sb, pa1_sb, tmn_sb, tm02_sb, mask_exp], axis=1)
    in_maps = []
    for l in range(L):
        aux = np.ascontiguousarray(
            np.concatenate([tta[l], aux_common], axis=1).astype(np.float32))
        in_maps.append({
            "lhs": np.ascontiguousarray(lhs[l]),
            "rhs": np.ascontiguousarray(rhs[l]),
            "aux": aux,
        })
    return in_maps


# --------------------------------------------------------------------------
# device program
# --------------------------------------------------------------------------

def _build_nc():
    import concourse.bacc as bacc
    import concourse.mybir as mybir
    import concourse.bass as bass
    from concourse.tile import TileContext

    f32 = mybir.dt.float32
    bf16 = mybir.dt.bfloat16
    f16 = mybir.dt.float16
    Alu = mybir.AluOpType
    Act = mybir.ActivationFunctionType

    nc = bacc.Bacc("TRN2", target_bir_lowering=False)
    lhs = nc.dram_tensor("lhs", [4 * KF, B * N], f16, kind="ExternalInput")
    rhs = nc.dram_tensor("rhs", [4 * KF, B * N], f16, kind="ExternalInput")
    aux = nc.dram_tensor("aux", [128, 2272], f32, kind="ExternalInput")
    out = nc.dram_tensor("out", [128, 8], f32, kind="ExternalOutput")

    with TileContext(nc) as tc:
        with (
            tc.tile_pool(name="const", bufs=1) as cp,
            tc.tile_pool(name="work", bufs=4) as wp,
            tc.tile_pool(name="dump", bufs=2) as dp,
            tc.tile_pool(name="psum", bufs=2, space="PSUM") as pp,
        ):
            lhs_sb = cp.tile([4 * KF, B * N], f16)
            rhs_sb = cp.tile([4 * KF, B * N], f16)
            # ordered by first use: minimal-count big DMAs
            nc.sync.dma_start(lhs_sb[:, 0:256], lhs[:, 0:256])
            nc.sync.dma_start(rhs_sb[:, 0:N], rhs[:, 0:N])
            nc.sync.dma_start(lhs_sb[:, 256:N], lhs[:, 256:N])
            aux_sb = cp.tile([128, 2272], f32)
            nc.sync.dma_start(aux_sb[:], aux[:])
            nc.sync.dma_start(lhs_sb[:, N:B * N], lhs[:, N:B * N])
            nc.sync.dma_start(rhs_sb[:, N:B * N], rhs[:, N:B * N])
            tta_sb = aux_sb[:, 0:448]
            tt_sb = aux_sb[:, 448:896]
            ta_sb = aux_sb[:, 896:1344]
            pt1_sb = aux_sb[:, 1344:1568]
            pa1_sb = aux_sb[:, 1568:1792]
            tmn_sb = aux_sb[:, 1792:2016]
            tm02_sb = aux_sb[:, 2016:2240]
            mask_sb = aux_sb[:, 2240:2272]

            acc = cp.tile([128, 32], f32)
            wsum = cp.tile([128, 8], f32)
            consts = cp.tile([128, 3], f32)
            nc.vector.memset(consts[:, 0:1], float(EPS + GUARD))
            nc.vector.memset(consts[:, 1:2], 1e-8)
            nc.vector.memset(consts[:, 2:3], -1.0)
            b_guard = consts[:, 0:1]
            b_eps8 = consts[:, 1:2]
            b_neg1 = consts[:, 2:3]

            lhs_v = lhs_sb[:].rearrange("k (b i p) -> k b i p", b=B, i=NT)
            rhs_v = rhs_sb[:].rearrange("k (b j n) -> k b j n", b=B, j=NJ)

            # ---- torsion ----
            tp = cp  # persistent intermediates
            sq = tp.tile([128, 448], f32)
            nc.vector.tensor_mul(sq[:], tta_sb, tta_sb)
            n2 = tp.tile([128, 224], f32)
            nc.vector.tensor_reduce(
                n2[:], sq[:].rearrange("p (a c) -> p a c", c=2),
                mybir.AxisListType.X, Alu.add)
            norm = tp.tile([128, 224], f32)
            nc.scalar.activation(norm[:], n2[:], Act.Sqrt, bias=b_eps8)
            rn = tp.tile([128, 224], f32)
            nc.vector.reciprocal_approx_fast(rn[:], norm[:])
            rn_bc = bass.AP(rn.tensor, rn.offset, [rn.ap[0], [1, 224], [0, 2]])
            unit = tp.tile([128, 448], f32)
            nc.vector.tensor_tensor(
                unit[:].rearrange("p (a c) -> p a c", c=2),
                tta_sb.rearrange("p (a c) -> p a c", c=2),
                rn_bc, Alu.mult)

            prodT = tp.tile([128, 448], f32)
            nc.vector.tensor_mul(prodT[:], tt_sb, unit[:])
            qT = tp.tile([128, 224], f32)
            nc.vector.tensor_reduce(
                qT[:], prodT[:].rearrange("p (a c) -> p a c", c=2),
                mybir.AxisListType.X, Alu.add)
            dT = tp.tile([128, 224], f32)
            nc.vector.scalar_tensor_tensor(
                dT[:], qT[:], -2.0, pt1_sb, Alu.mult, Alu.add)

            prodA = tp.tile([128, 448], f32)
            nc.vector.tensor_mul(prodA[:], ta_sb, unit[:])
            qA = tp.tile([128, 224], f32)
            nc.vector.tensor_reduce(
                qA[:], prodA[:].rearrange("p (a c) -> p a c", c=2),
                mybir.AxisListType.X, Alu.add)
            dA = tp.tile([128, 224], f32)
            nc.vector.scalar_tensor_tensor(
                dA[:], qA[:], -2.0, pa1_sb, Alu.mult, Alu.add)

            dmin = tp.tile([128, 224], f32)
            nc.vector.tensor_tensor(dmin[:], dT[:], dA[:], Alu.min)
            r1 = tp.tile([128, 224], f32)
            nc.vector.tensor_mul(r1[:], dmin[:], tmn_sb)
            d1 = tp.tile([128, 224], f32)
            nc.vector.tensor_scalar_add(d1[:], norm[:], -1.0)
            nl = tp.tile([128, 224], f32)
            nc.vector.scalar_tensor_tensor(
                nl[:], d1[:], -1.0, d1[:], Alu.mult, Alu.max)
            r2 = tp.tile([128, 224], f32)
            nc.vector.tensor_mul(r2[:], nl[:], tm02_sb)
            r3 = tp.tile([128, 224], f32)
            nc.vector.tensor_add(r3[:], r1[:], r2[:])
            nc.vector.tensor_reduce(
                wsum[:, 4:8], r3[:].rearrange("p (b a) -> p b a", b=B),
                mybir.AxisListType.X, Alu.add)

            # ---- FAPE main loop ----
            def acc_col(b, it):
                return b * 8 + it

            def fape_group(b, its):
                width = len(its) * 1024
                ps = pp.tile([128, width], f32, tag="ps", name=f"ps_{b}_{its[0]}")
                for k, it in enumerate(its):
                    for jh in range(NJ):
                        nc.tensor.matmul(
                            ps[:, (k * 2 + jh) * 512:(k * 2 + jh + 1) * 512],
                            lhs_v[:, b, it, :], rhs_v[:, b, jh, :],
                            start=True, stop=True)
                dist = wp.tile([128, width], bf16, tag="dist",
                               name=f"dist_{b}_{its[0]}")
                nc.scalar.activation(dist[:], ps[:], Act.Sqrt, bias=b_guard)
                for k, it in enumerate(its):
                    dump = dp.tile([128, 1024], bf16, tag="dump",
                                   name=f"dump_{b}_{it}")
                    nc.vector.tensor_scalar(
                        dump[:], dist[:, k * 1024:(k + 1) * 1024],
                        float(D_CLAMP), None,
                        Alu.min, Alu.add, accum_out=acc[:, acc_col(b, it):acc_col(b, it) + 1])

            for b in range(B):
                if b == 0:
                    groups = [[0], [1, 2], [3, 4], [5, 6], [7]]
                elif b == B - 1:
                    groups = [[0, 1], [2, 3], [4, 5], [6], [7]]
                else:
                    groups = [[0, 1], [2, 3], [4, 5], [6, 7]]
                for its in groups:
                    fape_group(b, its)
                # per-b weighted partition sum as soon as this b's cols land
                wmul = wp.tile([128, 8], f32, tag="wmul")
                nc.vector.tensor_mul(wmul[:], acc[:, b * 8:(b + 1) * 8],
                                     mask_sb[:, b * 8:(b + 1) * 8])
                nc.vector.tensor_reduce(
                    wsum[:, b:b + 1], wmul[:], mybir.AxisListType.X, Alu.add)

            # partition-dim reduction done on host (128x8 values)
            nc.sync.dma_start(out[:], wsum[:])

    nc.compile()
    return nc


# --------------------------------------------------------------------------
# host reference fallback (only used when seq_mask has zeros)
# --------------------------------------------------------------------------

def _numpy_reference(traj_rotations, traj_translations, traj_torsion_angles,
                     true_rotations, true_translations, true_torsion_angles,
                     true_torsion_angles_alt, res_types, seq_mask):
    f = np.float32
    Rt_inv = np.swapaxes(true_rotations, -1, -2)
    tt_inv = -np.einsum('birc,bic->bir', Rt_inv, true_translations)
    x_true = np.einsum('biop,bjp->bijo', Rt_inv, true_translations) + tt_inv[:, :, None, :]
    Rp_inv = np.swapaxes(traj_rotations, -1, -2)
    tp_inv = -np.einsum('lbirc,lbic->lbir', Rp_inv, traj_translations)
    x_pred = np.einsum('lbiop,lbjp->lbijo', Rp_inv, traj_translations) + tp_inv[:, :, :, None, :]
    dist = np.sqrt(np.sum((x_pred - x_true[None]) ** 2, -1) + EPS)
    dist = np.minimum(dist, D_CLAMP)
    pm = seq_mask[:, :, None] * seq_mask[:, None, :]
    pc = np.maximum(pm.sum((-1, -2)), 1.0)
    fape = (1.0 / Z) * np.sum(dist * pm[None], (-1, -2)) / pc
    norm = np.sqrt(np.sum(traj_torsion_angles ** 2, -1) + 1e-8)
    unit = traj_torsion_angles / norm[..., None]
    d_true = np.sum((true_torsion_angles[None] - unit) ** 2, -1)
    d_alt = np.sum((true_torsion_angles_alt[None] - unit) ** 2, -1)
    dsq = np.minimum(d_true, d_alt)
    chi = CHI_MASK_TABLE[res_types].astype(f)
    tmask = np.concatenate([np.ones_like(chi[..., :3]), chi], -1) * seq_mask[..., None]
    normalizer = np.maximum(tmask.sum((1, 2)), 1.0)
    tl = np.sum(dsq * tmask[None], (2, 3)) / normalizer
    anl = np.sum(np.abs(norm - 1.0) * tmask[None], (2, 3)) / normalizer
    return (np.sum(fape + tl + 0.02 * anl, 0) / L).astype(f)


# --------------------------------------------------------------------------
# entry point
# --------------------------------------------------------------------------

def kernel(**inputs):
    global LAST_RESULTS
    inputs = {k: np.asarray(v) for k, v in inputs.items()}
    seq_mask = inputs["seq_mask"].astype(np.float32)
    if not np.all(seq_mask == 1.0):
        # general-mask fallback (never hit for the benchmark distribution,
        # where seq_mask is all ones)
        return _numpy_reference(**inputs)

    in_maps = _build_host_data(**inputs)

    if "nc" not in _NC_CACHE:
        _NC_CACHE["nc"] = _build_nc()
    nc = _NC_CACHE["nc"]

    import os
    from concourse.bass_utils import run_bass_kernel_spmd
    trace = bool(int(os.environ.get("KERNEL_TRACE", "0")))
    try:
        res = run_bass_kernel_spmd(nc, in_maps, core_ids=list(range(L)), trace=trace)
    except Exception:
        # transient runtime/device-state hiccups: retry once
        res = run_bass_kernel_spmd(nc, in_maps, core_ids=list(range(L)), trace=trace)
    LAST_RESULTS = res

    outs = np.stack([r["out"].astype(np.float64).sum(0) for r in res.results])  # (L, 8)
    return (outs[:, 0:4].sum(0) + outs[:, 4:8].sum(0)).astype(np.float32)
